# revision 50
# baseline (speedup 1.0000x reference)
"""Bass/Trainium2 kernel for LlamaAttention (GQA + RoPE + RMSNorm + causal attention).

Sharding: tensor-parallel over heads across 8 NeuronCores.
  core m: wq columns [m*4h], wk/wv columns [m*1 kv head], wo rows [m*512:...]
  fp8 avn AllGathered across cores; each core computes a wo column slice.

Layout strategy (per core):
  - x shipped pre-transposed/pre-tiled as bf16 -> hT streams as matmul rhs;
    next chunk's window prefetched during attention (ahead of the RS window)
  - q/k produced TRANSPOSED ([dim, token]) straight from weight-stationary
    matmuls; RMSNorm scale s(t) folded into RoPE cos/sin (and into v directly);
    s broadcast across partitions via a K=1 ones-matmul on the PE (no DMA)
  - scoresT[kt,qt] = kT.T @ qT -> exp -> probsT ready as `av` matmul rhs
  - softmax denominators split across engines: even k-blocks summed on the PE
    (PSUM row accumulate), odd blocks on Vector, folded by one f32 matmul;
    reciprocal broadcast via a K=1 ones-matmul on the PE (a DMA bounce here
    would run under the previous chunk's RS and stall the vector queue)
  - wo runs in fp8e4 (e4m3) with DoubleRow perf mode: avn written as
    fp8 (pre-scaled by AVC via the den reciprocal), wo host-quantized with a
    per-tensor scale; dequant is folded into the output residual-add.
    fp8 is confined to wo: quantizing q/k/v pushes rel err past the 2e-2 gate.
  - collective is an AllGather of the fp8 avn (0.25MB/chunk, two head-waves)
    instead of a ReduceScatter of bf16 partials (4MB/chunk); wo is then
    column-sharded per core with full-K f32 PSUM accumulation. The gather
    wait lives on the Pool queue only, and chunk c's wo matmuls are emitted
    after proj(c+1) so their AG dependency has long resolved: no compute
    queue ever head-blocks on collective latency (on the RS design this
    stalled every chunk boundary ~40us).
"""
import sys, math, os

for p in ("/opt/trn_rl_repo", "/root/.axon_site/_ro/trn_rl_repo"):
    if os.path.isdir(p) and p not in sys.path:
        sys.path.insert(0, p)

import numpy as np
import ml_dtypes

import concourse.bass as bass
import concourse.tile as tile
from concourse import bacc, mybir
from concourse.bass_utils import run_bass_kernel_spmd

bf16 = ml_dtypes.bfloat16
F32 = mybir.dt.float32
F32R = mybir.dt.float32r
I32 = mybir.dt.int32
BF16 = mybir.dt.bfloat16
FP8 = mybir.dt.float8e4
fp8np = mybir.dt.np(mybir.dt.float8e4)
Act = mybir.ActivationFunctionType

NCORES = 8
DH = 128          # head dim
EPS = 1e-5
CH = 512          # token chunk
AVC = 32.0        # fp8 scale for normalized attention outputs (absmax ~4.5)


def build(T, D, QH, s_wo):
    """Build the SPMD Bass program. T tokens, D model dim, QH local q heads.
    s_wo: host-side fp8 quant scale of wo (dequant folded into the output stt)."""
    KD = D // 128           # k-chunks over model dim
    NCH = T // CH           # token chunks
    MCH = D // 512          # output-column chunks of wo matmul
    SC = 1.0 / math.sqrt(DH)
    DEQ = 1.0 / (AVC * s_wo)

    nc = bacc.Bacc("TRN2", target_bir_lowering=False, debug=False, num_devices=NCORES)

    # ---- DRAM parameters (per-core shards / replicated) ----
    xarr = nc.dram_tensor("xarr", [128, NCH * KD * 512], BF16, kind="ExternalInput").ap()
    cosP = nc.dram_tensor("cosP", [128, T], F32, kind="ExternalInput").ap()
    sinP = nc.dram_tensor("sinP", [128, T], F32, kind="ExternalInput").ap()
    wqp = nc.dram_tensor("wqp", [128, KD * QH * 128], BF16, kind="ExternalInput").ap()
    wkp = nc.dram_tensor("wkp", [128, KD * 128], BF16, kind="ExternalInput").ap()
    wvp = nc.dram_tensor("wvp", [128, KD * 128], BF16, kind="ExternalInput").ap()
    # wo COLUMN slice for this core: [dh, wave, src_core, h_in_pair, col]
    wop = nc.dram_tensor("wop", [128, 2, NCORES, 2, D // NCORES], FP8,
                         kind="ExternalInput").ap()
    # residual column slice x[:, m*512:(m+1)*512]
    xres = nc.dram_tensor("xres", [T, D // NCORES], F32, kind="ExternalInput").ap()
    tri = nc.dram_tensor("tri", [128, 128], BF16, kind="ExternalInput").ap()
    ident = nc.dram_tensor("ident", [128, 128], BF16, kind="ExternalInput").ap()
    onescol = nc.dram_tensor("onescol", [128, 1], BF16, kind="ExternalInput").ap()
    outp = nc.dram_tensor("out", [T, D // NCORES], F32, kind="ExternalOutput").ap()

    # AllGather-of-avn instead of ReduceScatter-of-partials: ships the fp8
    # avn (0.25MB/chunk) instead of bf16 partials (4MB/chunk); wo becomes
    # column-sharded with full-K f32 PSUM accumulation (no bf16 partial
    # rounding). Head-waves per chunk so the tail only waits for the last
    # head's gather — single-head waves on the last chunk halve the final
    # transfer (the gathered heads land in the two planes of one paired
    # SBUF tile, so the wo DoubleRow pairing is unaffected).
    WAVES = [[(0, 1), (2,), (3,)] for _ in range(NCH)]
    avd = [[nc.dram_tensor(f"avd_{c}_{wi}", [128, len(ws), CH], FP8)
            for wi, ws in enumerate(WAVES[c])] for c in range(NCH)]
    agd = [[nc.dram_tensor(f"agd_{c}_{wi}", [NCORES * 128, len(ws), CH], FP8,
                           addr_space="Shared")
            for wi, ws in enumerate(WAVES[c])] for c in range(NCH)]

    with tile.TileContext(nc) as tc:
        import contextlib
        ctx = contextlib.ExitStack()
        with ctx:
            cpool = ctx.enter_context(tc.tile_pool(name="consts", bufs=1))
            wpool = ctx.enter_context(tc.tile_pool(name="weights", bufs=1))
            xpool = ctx.enter_context(tc.tile_pool(name="xw", bufs=1))
            cs = ctx.enter_context(tc.tile_pool(name="cs", bufs=1))
            work = ctx.enter_context(tc.tile_pool(name="work", bufs=2))
            x2p = ctx.enter_context(tc.tile_pool(name="x2p", bufs=5))
            kvp = ctx.enter_context(tc.tile_pool(name="kvp", bufs=1))
            prp = ctx.enter_context(tc.tile_pool(name="prp", bufs=3))
            evp = ctx.enter_context(tc.tile_pool(name="evp", bufs=3))
            finp = ctx.enter_context(tc.tile_pool(name="finp", bufs=2))
            ps = ctx.enter_context(tc.tile_pool(name="ps", bufs=1, space="PSUM"))

            # ---- constants resident; chunk-0 activations issued BEFORE the
            # 22MB of weights so the PE isn't starved at kernel start ----
            tri_sb = cpool.tile([128, 128], BF16)
            id_sb = cpool.tile([128, 128], BF16)
            ones_sb = cpool.tile([128, 1], BF16)
            # den accumulators use value 1/AVC so the reciprocal comes out as
            # AVC/den, pre-scaling avn into fp8 range at zero extra cost
            onesd_sb = cpool.tile([128, 1], BF16)
            onesd_f32 = cpool.tile([128, 1], F32R)
            onesrow = cpool.tile([1, 128], F32)
            onesrow_r = cpool.tile([1, 128], F32R)
            eps_row = cpool.tile([1, 512], F32)
            zeros_sb = cpool.tile([128, 128], F32)
            ones_dr = cpool.tile([128, 2, 32], FP8)
            nc.vector.memset(ones_dr, 1.0)
            nc.vector.memset(zeros_sb, 0.0)
            nc.vector.memset(onesd_sb, 1.0 / AVC)
            nc.vector.memset(onesrow, 1.0)
            nc.vector.memset(eps_row, EPS)
            # memset can't target f32r; fill via dtype-converting copies
            nc.vector.tensor_copy(onesd_f32, onesd_sb)
            nc.vector.tensor_copy(onesrow_r, onesrow)
            nc.sync.dma_start(out=tri_sb, in_=tri)
            nc.sync.dma_start(out=id_sb, in_=ident)
            nc.sync.dma_start(out=ones_sb, in_=onescol)

            KH = KD // 2  # half window of k-chunks
            # first k-pair split out so the ssq chain starts ~5us earlier
            xw0A = xpool.tile([128, KH * 512], BF16, tag="xwA")
            xw0B = xpool.tile([128, KH * 512], BF16, tag="xwB")
            nc.sync.dma_start(out=xw0A[:, 0:1024], in_=xarr[:, 0:1024])
            nc.sync.dma_start(out=xw0A[:, 1024:KH * 512], in_=xarr[:, 1024:KH * 512])
            nc.sync.dma_start(out=xw0B, in_=xarr[:, KH * 512:KD * 512])
            cos0 = cs.tile([128, 512], F32, tag="cos")
            sin0 = cs.tile([128, 512], F32, tag="sin")
            nc.sync.dma_start(out=cos0, in_=cosP[:, 0:CH])
            nc.sync.dma_start(out=sin0, in_=sinP[:, 0:CH])

            # wq is packed per-head contiguous; load head-by-head (1MB each) so
            # head 0's projection can start ~10us in, before wk/wv/wo arrive
            wq_sb = wpool.tile([128, QH * KD * 128], BF16)
            wk_sb = wpool.tile([128, KD * 128], BF16)
            wv_sb = wpool.tile([128, KD * 128], BF16)
            woc_sb = wpool.tile([128, 2, NCORES, 2, D // NCORES], FP8)
            HW = KD * 128
            for h in range(QH):
                nc.sync.dma_start(out=wq_sb[:, h * HW:(h + 1) * HW],
                                  in_=wqp[:, h * HW:(h + 1) * HW])
            nc.sync.dma_start(out=wk_sb, in_=wkp)
            nc.sync.dma_start(out=wv_sb, in_=wvp)
            nc.sync.dma_start(out=woc_sb, in_=wop)

            # residual column-slice tiles: loaded per chunk (consumed by
            # wo_block(c) during chunk c+1), rotating 2 buffers per tb slot
            xr_t = {}

            def load_xr(cc):
                for tb in range(4):
                    i = cc * 4 + tb
                    xr_sb = finp.tile([128, D // NCORES], F32, tag=f"xr{tb}",
                                      bufs=2)
                    nc.sync.dma_start(out=xr_sb, in_=xres[i * 128:(i + 1) * 128, :])
                    xr_t[(cc, tb)] = xr_sb

            # persistent k/v for the whole sequence
            kT_sb = kvp.tile([128, T], BF16)   # [dim, token], roped+normed
            v_sb = kvp.tile([128, T], BF16)    # v natural: block j at cols j*128

            def prefetch_x(c):
                """Issue chunk c's x-window/cos/sin DMAs. Called one chunk
                ahead (right after proj(c-1)) so the 4.2MB window is resident
                before the previous chunk's ReduceScatter contends for DMA."""
                csl = bass.ds(c * CH, CH)
                xwA = xpool.tile([128, KH * 512], BF16, tag="xwA")
                xwB = xpool.tile([128, KH * 512], BF16, tag="xwB")
                nc.sync.dma_start(out=xwA, in_=xarr[:, (c * KD) * 512:(c * KD + KH) * 512])
                nc.sync.dma_start(out=xwB, in_=xarr[:, (c * KD + KH) * 512:(c * KD + KD) * 512])
                cos_sl = cs.tile([128, 512], F32, tag="cos")
                sin_sl = cs.tile([128, 512], F32, tag="sin")
                nc.sync.dma_start(out=cos_sl, in_=cosP[:, csl])
                nc.sync.dma_start(out=sin_sl, in_=sinP[:, csl])
                return xwA, xwB, cos_sl, sin_sl

            def norm_rest(c, tiles):
                """rmsnorm scale + rope tables for chunk c (at chunk start)."""
                xwA, xwB, cos_sl, sin_sl = tiles

                def xw(k):
                    buf = xwA if k < KH else xwB
                    kk = k % KH
                    return buf[:, kk * 512:(kk + 1) * 512]

                # x^2 in fp8e4 so the ssq reduction runs DoubleRow (0.5 cyc/row
                # instead of 1): x^2 <= ~30 fits e4m3 easily, and the per-term
                # ~4% quantization noise averages to <0.1% over the 4096-sum.
                # Engines still alternate (vector even plane, scalar odd).
                ssq_ps = ps.tile([32, 512], F32, tag="row", bufs=1)
                for i in range(KD // 2):
                    x2 = x2p.tile([128, 2, 512], FP8, tag="x2")
                    nc.vector.tensor_mul(x2[:, 0, :], xw(2 * i), xw(2 * i))
                    if c == 0 and i == 0:
                        # keep Scalar's one-time ACT_TABLE load off the
                        # kernel-start critical path
                        nc.vector.tensor_mul(x2[:, 1, :], xw(1), xw(1))
                    else:
                        nc.scalar.activation(x2[:, 1, :], xw(2 * i + 1), Act.Square)
                    nc.tensor.matmul(ssq_ps, ones_dr, x2,
                                     start=(i == 0), stop=(i == KD // 2 - 1),
                                     perf_mode=mybir.MatmulPerfMode.DoubleRow)
                # rsqrt entirely on DVE (magic-constant seed + 2 Newton steps).
                # Keeps Sqrt off the Scalar engine: sqrt lives in a different
                # activation-table set than exp/square/copy, so using it forced
                # two ~1.3us ACT_TABLE_LOADs per chunk right in the exp path.
                # The seed constant math runs in fp32 VALUE space (DVE add/mult
                # fp-cast int operands anyway); 2^-24 rounding on the int is
                # far below the seed's own 3.4% error.
                z_sb = work.tile([1, 512], F32, tag="sq", bufs=1)
                nc.vector.scalar_tensor_tensor(
                    out=z_sb[0:1, :], in0=ssq_ps[0:1, :], scalar=1.0 / D,
                    in1=eps_row[0:1, :], op0=mybir.AluOpType.mult,
                    op1=mybir.AluOpType.add)
                # s_sb is float32r so its producer "rounds to FP32r" (BIR
                # verifier requirement for f32r matmul operands)
                s_sb = work.tile([1, 512], F32R, tag="s", bufs=1)
                u_sb = work.tile([1, 512], F32, tag="u", bufs=1)
                w_sb = work.tile([1, 512], I32, tag="wi", bufs=1)
                # u = float(bits(z)); u = C - 0.5*u (seed bits as float value);
                # w = int(u); s = bitcast(w). s_sb only ever sees f32r-typed
                # writes (BIR verifier: f32r matmul operands need f32r writers)
                nc.vector.tensor_copy(u_sb[0:1, :], z_sb[0:1, :].bitcast(I32))
                nc.vector.tensor_scalar(
                    out=u_sb[0:1, :], in0=u_sb[0:1, :], scalar1=-0.5,
                    scalar2=float(0x5F3759DF), op0=mybir.AluOpType.mult,
                    op1=mybir.AluOpType.add)
                nc.vector.tensor_copy(w_sb[0:1, :], u_sb[0:1, :])
                nc.vector.tensor_copy(s_sb[0:1, :], w_sb[0:1, :].bitcast(F32))
                for _ in range(2):
                    nc.vector.tensor_mul(u_sb[0:1, :], s_sb[0:1, :], s_sb[0:1, :])
                    nc.vector.tensor_mul(u_sb[0:1, :], u_sb[0:1, :], z_sb[0:1, :])
                    nc.vector.tensor_scalar(
                        out=u_sb[0:1, :], in0=u_sb[0:1, :], scalar1=-0.5,
                        scalar2=1.5, op0=mybir.AluOpType.mult,
                        op1=mybir.AluOpType.add)
                    nc.vector.tensor_mul(s_sb[0:1, :], s_sb[0:1, :], u_sb[0:1, :])
                # broadcast partition 0 -> 128 ON THE PE (K=1 ones-matmul into
                # PSUM): no DMA involved, so the chunk-start rope chain never
                # races a ReduceScatter for DMA bandwidth. f32r: 1 cyc/row on
                # the PE instead of fp32's 4.
                s_ps = ps.tile([128, 512], F32, tag="sbc", bufs=1)
                nc.tensor.matmul(s_ps, onesrow_r[0:1, :], s_sb[0:1, :],
                                 start=True, stop=True)
                s_bc = work.tile([128, 512], F32, tag="s_bc")
                nc.vector.tensor_copy(s_bc, s_ps)

                cosn = cs.tile([128, 512], BF16, tag="cosn")
                sinn = cs.tile([128, 512], BF16, tag="sinn")
                nc.vector.tensor_mul(cosn, cos_sl, s_ps)
                nc.vector.tensor_mul(sinn, sin_sl, s_ps)
                return xw, s_bc, cosn, sinn

            avf_t = {}

            def wo_block(cc, interleave=False):
                """Column-sharded wo for chunk cc from the gathered fp8 avn:
                16 DoubleRow matmuls per 128-token block (full K=4096 in f32
                PSUM), then one fused dequant+residual stt and the store.
                interleave=True (last chunk): run every block's wave-0 half
                first so the PE chews through it while AG(wave 1) is still in
                flight — needs 4 concurrent PSUM groups (3 acc + 1 scores)."""
                def mm(wo_ps, w, j, tb, start, stop):
                    nc.tensor.matmul(
                        wo_ps,
                        avf_t[(w, j)][0][:, :, tb * 128:(tb + 1) * 128],
                        woc_sb[:, w, j, :, :],
                        start=start, stop=stop,
                        perf_mode=mybir.MatmulPerfMode.DoubleRow)

                def evac(wo_ps, tb):
                    o_sb = evp.tile([128, 512], F32, tag="osb")
                    i = cc * 4 + tb
                    nc.vector.scalar_tensor_tensor(
                        out=o_sb, in0=wo_ps, scalar=DEQ, in1=xr_t[(cc, tb)],
                        op0=mybir.AluOpType.mult, op1=mybir.AluOpType.add)
                    nc.gpsimd.dma_start(out=outp[i * 128:(i + 1) * 128, :],
                                        in_=o_sb)

                if not interleave:
                    for tb in range(4):
                        wo_ps = ps.tile([128, 512], F32, tag="acc", bufs=3)
                        for n in range(2 * NCORES):
                            mm(wo_ps, n // NCORES, n % NCORES, tb,
                               n == 0, n == 2 * NCORES - 1)
                        evac(wo_ps, tb)
                else:
                    psums = []
                    for tb in range(4):
                        wo_ps = ps.tile([128, 512], F32,
                                        tag="acc" if tb < 3 else "scores",
                                        bufs=3 if tb < 3 else 2)
                        psums.append(wo_ps)
                        for j in range(NCORES):
                            mm(wo_ps, 0, j, tb, j == 0, False)
                    for tb in range(4):
                        for j in range(NCORES):
                            mm(psums[tb], 1, j, tb, False, j == NCORES - 1)
                        evac(psums[tb], tb)

            nxt_x = (xw0A, xw0B, cos0, sin0)

            for c in range(NCH):
                csl = bass.ds(c * CH, CH)
                xw, s_bc, cosn, sinn = norm_rest(c, nxt_x)

                # ---- projections (weight-stationary, transposed outputs) ----
                def rope_evac(pp, dest):
                    # dest = pp * cosn + swap64(pp) * sinn   (sign baked into sinn)
                    t1 = work.tile([128, 512], F32, tag="t1")
                    t2 = work.tile([128, 512], F32, tag="t2")
                    nc.vector.tensor_mul(t1, pp, cosn)
                    nc.vector.tensor_mul(t2[0:64, :], pp[64:128, :], sinn[0:64, :])
                    nc.vector.tensor_mul(t2[64:128, :], pp[0:64, :], sinn[64:128, :])
                    nc.vector.tensor_add(dest, t1, t2)

                qT_all = work.tile([128, QH * 512], BF16, tag="qT")
                for h in range(QH):
                    pp = ps.tile([128, 512], F32, tag="acc", bufs=3)
                    for k in range(KD):
                        nc.tensor.matmul(pp, wq_sb[:, (h * KD + k) * 128:(h * KD + k + 1) * 128],
                                         xw(k), start=(k == 0), stop=(k == KD - 1))
                    rope_evac(pp, qT_all[:, h * 512:(h + 1) * 512])
                pp = ps.tile([128, 512], F32, tag="acc", bufs=3)
                for k in range(KD):
                    nc.tensor.matmul(pp, wk_sb[:, k * 128:(k + 1) * 128], xw(k),
                                     start=(k == 0), stop=(k == KD - 1))
                rope_evac(pp, kT_sb[:, csl])
                pp = ps.tile([128, 512], F32, tag="acc", bufs=3)
                for k in range(KD):
                    nc.tensor.matmul(pp, wv_sb[:, k * 128:(k + 1) * 128], xw(k),
                                     start=(k == 0), stop=(k == KD - 1))
                vtmp = work.tile([128, 512], BF16, tag="vtmp")
                nc.vector.tensor_mul(vtmp, pp, s_bc)
                for tb in range(4):
                    trp = ps.tile([128, 128], BF16, tag="tr", bufs=1)
                    nc.tensor.transpose(trp, vtmp[:, tb * 128:(tb + 1) * 128], id_sb)
                    nc.vector.tensor_copy(v_sb[:, (4 * c + tb) * 128:(4 * c + tb + 1) * 128], trp)

                # prefetch next chunk's x window now (its WAR on proj(c) clears
                # exactly as proj drains; arrives during attention, before the
                # AG window could starve it)
                if c + 1 < NCH:
                    nxt_x = prefetch_x(c + 1)
                load_xr(c)

                # ---- wo for the PREVIOUS chunk: emitted here (after proj(c),
                # before attention(c)) so its PE-queue wait on AG(c-1) has long
                # resolved by the time the PE reaches it ----
                if c > 0:
                    wo_block(c - 1)

                # ---- attention for this chunk's 512 query tokens ----
                NJ = 4 * (c + 1)
                avn_all = work.tile([128, QH, 512], FP8, tag="avn")
                for h in range(QH):
                    av_ps = ps.tile([128, 512], F32, tag="acc", bufs=3)
                    # den partials accumulate OFF the PE entirely: odd j on
                    # Vector (plane 0), even j on the idle Pool engine
                    # (plane 1) — independent chains, no cross-engine
                    # ping-pong — folded by two f32r matmuls at the end.
                    den_ps = ps.tile([1, 512], F32, tag="row", bufs=1)
                    den_acc = work.tile([128, 2, 512], F32R, tag="den_acc", bufs=1)
                    if c == 0:
                        # first blocks are diagonal: zero the never-written
                        # [0:off] region of each plane's first write
                        nc.vector.tensor_copy(den_acc[:, 0, 0:128], zeros_sb)
                    qTh = qT_all[:, h * 512:(h + 1) * 512]
                    for j in range(NJ):
                        sc_ps = ps.tile([128, 512], F32, tag="scores", bufs=2)
                        nc.tensor.matmul(sc_ps, kT_sb[:, j * 128:(j + 1) * 128], qTh,
                                         start=True, stop=True)
                        pr = prp.tile([128, 512], BF16, tag="pr")
                        off = max(0, 128 * (j - 4 * c))
                        nc.scalar.activation(pr[:, off:512], sc_ps[:, off:512],
                                             Act.Exp, scale=SC)
                        if j >= 4 * c:
                            nc.vector.tensor_mul(pr[:, off:off + 128], pr[:, off:off + 128], tri_sb)
                        eng = nc.gpsimd if j % 2 == 0 else nc.vector
                        pl = den_acc[:, 1 - j % 2, :]
                        if j < 2:
                            eng.tensor_copy(pl[:, off:512], pr[:, off:512])
                        else:
                            eng.tensor_add(pl[:, off:512], pl[:, off:512],
                                           pr[:, off:512])
                        nc.tensor.matmul(av_ps[:, off:512], v_sb[:, j * 128:(j + 1) * 128],
                                         pr[:, off:512], start=(j == 0), stop=(j == NJ - 1))
                    # fold both planes into the PSUM row (f32r: 1 cyc/row)
                    nc.tensor.matmul(den_ps, onesd_f32, den_acc[:, 1, :],
                                     start=True, stop=False)
                    nc.tensor.matmul(den_ps, onesd_f32, den_acc[:, 0, :],
                                     start=False, stop=True)
                    den_r = work.tile([1, 512], F32, tag="den_r")
                    nc.vector.reciprocal_approx_fast(out=den_r[0:1, :], in_=den_ps[0:1, :])
                    # broadcast the reciprocal on the PE (no DMA bounce). The
                    # f32r round-trip (reciprocal must write f32) costs one
                    # small DVE copy but drops the matmul from 4 cyc/row to 1.
                    den_rr = work.tile([1, 512], F32R, tag="den_rr", bufs=1)
                    nc.vector.tensor_copy(den_rr[0:1, :], den_r[0:1, :])
                    den_bc = ps.tile([128, 512], F32, tag="sbc", bufs=1)
                    nc.tensor.matmul(den_bc, onesrow_r[0:1, :], den_rr[0:1, :],
                                     start=True, stop=True)
                    avs = work.tile([128, 512], BF16, tag="avs")
                    nc.vector.tensor_copy(avs, av_ps)
                    nc.vector.tensor_mul(avn_all[:, h, :], avs, den_bc)

                    # ship completed waves: write fp8 avn to DRAM, AllGather,
                    # stage the 8 cores' slices back into paired SBUF tiles.
                    # Pool-queue ops (the gather wait never blocks compute
                    # queues); the final wave's loads split across Pool+Sync
                    # so their serial issue doesn't pace the tail.
                    for wi, ws in enumerate(WAVES[c]):
                        if h != ws[-1]:
                            continue
                        nc.sync.dma_start(
                            out=avd[c][wi].ap(),
                            in_=avn_all[:, ws[0]:ws[-1] + 1, :])
                        nc.gpsimd.collective_compute(
                            "AllGather", mybir.AluOpType.bypass,
                            replica_groups=[list(range(NCORES))],
                            ins=[avd[c][wi].ap()], outs=[agd[c][wi].ap()])
                        last_wave = wi == len(WAVES[c]) - 1
                        for j in range(NCORES):
                            for e, hh in enumerate(ws):
                                p = hh // 2
                                if (p, j) not in avf_t or avf_t[(p, j)][1] != c:
                                    avf_t[(p, j)] = (finp.tile(
                                        [128, 2, CH], FP8, tag=f"avf{p}_{j}",
                                        bufs=1, name=f"avf{p}_{j}"), c)
                                t = avf_t[(p, j)][0]
                                eng = nc.sync if (last_wave and c == NCH - 1
                                                  and j >= 4) else nc.gpsimd
                                eng.dma_start(
                                    out=t[:, hh % 2:hh % 2 + 1, :],
                                    in_=agd[c][wi].ap()[j * 128:(j + 1) * 128,
                                                        e:e + 1, :])

                if c == NCH - 1:
                    wo_block(c, interleave=True)

    nc.compile()
    return nc


# host-side permutation: de-interleave rope pairs (2i, 2i+1) -> (i, 64+i)
_PERM = np.concatenate([np.arange(0, DH, 2), np.arange(1, DH, 2)])


def wo_scale(wo):
    return 240.0 / max(float(np.abs(wo).max()), 1e-30)


def host_prep(x, r_cos, r_sin, w_norm, wq, wk, wv, wo, T, D, QH):
    """Build per-core input maps."""
    KD = D // 128
    NCH = T // CH
    NH = wq.shape[1] // DH
    NKV = wk.shape[1] // DH
    s_wo = wo_scale(wo)

    xT = np.ascontiguousarray(x.T)  # [D, T]
    xarr = np.ascontiguousarray(
        xT.reshape(KD, 128, NCH, 512).transpose(1, 2, 0, 3)).reshape(128, NCH * KD * 512)
    xarr = xarr.astype(bf16)

    cosP = np.ascontiguousarray(r_cos.T[_PERM, :]).astype(np.float32)
    sinP = np.ascontiguousarray(r_sin.T[_PERM, :]).astype(np.float32)
    sinP[:64, :] *= -1.0

    wn = w_norm[:, None].astype(np.float32)
    wq_p = (wq * wn).reshape(D, NH, DH)[:, :, _PERM].reshape(D, NH * DH)
    wk_p = (wk * wn).reshape(D, NKV, DH)[:, :, _PERM].reshape(D, NKV * DH)
    wv_p = wv * wn

    tri_m = (np.arange(128)[:, None] <= np.arange(128)[None, :]).astype(bf16)  # kt <= qt
    ident = np.eye(128, dtype=bf16)
    onescol = np.ones((128, 1), dtype=bf16)

    S = CH // NCORES  # 64 rows per core per chunk

    in_maps = []
    for m in range(NCORES):
        wq_m = wq_p[:, m * QH * 128:(m + 1) * QH * 128]
        # head-major packing: [128][QH][KD][128] so each head's weights are a
        # contiguous 1MB DMA
        wq_m = np.ascontiguousarray(
            wq_m.reshape(KD, 128, QH, 128).transpose(1, 2, 0, 3)).reshape(128, QH * KD * 128)
        wk_m = wk_p[:, m * 128:(m + 1) * 128]
        wk_m = np.ascontiguousarray(
            wk_m.reshape(KD, 128, 128).transpose(1, 0, 2)).reshape(128, KD * 128)
        wv_m = wv_p[:, m * 128:(m + 1) * 128]
        wv_m = np.ascontiguousarray(
            wv_m.reshape(KD, 128, 128).transpose(1, 0, 2)).reshape(128, KD * 128)
        # wo COLUMN slice for this core, rows regrouped as
        # [dh, wave, src_core, h_in_pair, col] to match the DR pairing of the
        # gathered avn (src core j's heads 2w/2w+1 are dim rows (j*4+2w+e)*128+dh)
        wo_m = wo[:, m * (D // NCORES):(m + 1) * (D // NCORES)]
        wo_m = np.ascontiguousarray(
            wo_m.reshape(NCORES, 2, 2, 128, D // NCORES).transpose(3, 1, 0, 2, 4))
        wo_m = np.clip(wo_m * s_wo, -240.0, 240.0).astype(fp8np)

        # residual column slice, rows already grouped per (chunk, token-block)
        xres_m = np.ascontiguousarray(
            x[:, m * (D // NCORES):(m + 1) * (D // NCORES)]).astype(np.float32)

        in_maps.append({
            "xarr": xarr, "cosP": cosP, "sinP": sinP,
            "wqp": wq_m.astype(bf16), "wkp": wk_m.astype(bf16),
            "wvp": wv_m.astype(bf16), "wop": wo_m,
            "xres": np.ascontiguousarray(xres_m),
            "tri": tri_m, "ident": ident, "onescol": onescol,
        })
    return in_maps


def assemble(results, T, D):
    # each core produced its full-height output column slice
    out = np.empty((T, D), np.float32)
    for m in range(NCORES):
        out[:, m * (D // NCORES):(m + 1) * (D // NCORES)] = results[m]["out"]
    return out


_CACHE = {}


def _get_nc(T, D, QH, s_wo):
    key = (T, D, QH, s_wo)
    if key not in _CACHE:
        _CACHE[key] = build(T, D, QH, s_wo)
    return _CACHE[key]


class Runner:
    """Cached-jit SPMD runner (replicates bass2jax.run_bass_via_pjrt but reuses the
    jitted callable across calls and supports device-resident inputs for timing)."""

    def __init__(self, nc, n_cores=NCORES):
        import jax
        from jax.experimental.shard_map import shard_map
        from jax.sharding import Mesh, PartitionSpec, NamedSharding
        from concourse import bass2jax
        bass2jax.install_neuronx_cc_hook()
        self.jax = jax
        self.nc = nc
        self.n_cores = n_cores
        partition_name = nc.partition_id_tensor.name if nc.partition_id_tensor else None
        in_names, out_names, out_avals, zero_shapes = [], [], [], []
        for alloc in nc.m.functions[0].allocations:
            if not isinstance(alloc, mybir.MemoryLocationSet):
                continue
            name = alloc.memorylocations[0].name
            if alloc.kind == "ExternalInput":
                if name != partition_name:
                    in_names.append(name)
            elif alloc.kind == "ExternalOutput":
                out_names.append(name)
                shape = tuple(alloc.tensor_shape)
                dtype = mybir.dt.np(alloc.dtype)
                out_avals.append(jax.core.ShapedArray(shape, dtype))
                zero_shapes.append((shape, dtype))
        self.in_names, self.out_names = in_names, out_names
        self.out_avals, self.zero_shapes = out_avals, zero_shapes
        n_params, n_outs = len(in_names), len(out_names)
        all_names = in_names + out_names
        if partition_name is not None:
            all_names = all_names + [partition_name]

        def _body(*args):
            operands = list(args)
            if partition_name is not None:
                operands.append(bass2jax.partition_id_tensor())
            outs = bass2jax._bass_exec_p.bind(
                *operands,
                out_avals=tuple(out_avals),
                in_names=tuple(all_names),
                out_names=tuple(out_names),
                lowering_input_output_aliases=(),
                sim_require_finite=True,
                sim_require_nnan=True,
                nc=nc,
            )
            return tuple(outs)

        devices = jax.devices()[:n_cores]
        self.mesh = Mesh(np.asarray(devices), ("core",))
        self.in_sharding = NamedSharding(self.mesh, PartitionSpec("core"))
        in_specs = (PartitionSpec("core"),) * (n_params + n_outs)
        out_specs = (PartitionSpec("core"),) * n_outs
        # No donation: the kernel writes every ExternalOutput element, so the
        # "zero" output operands can stay device-resident and be reused across
        # calls instead of being re-uploaded (32 MB/call through the tunnel).
        self.fn = jax.jit(
            shard_map(_body, mesh=self.mesh, in_specs=in_specs,
                      out_specs=out_specs, check_rep=False),
            keep_unused=True)
        self._dev_zeros = None

    def concat_inputs(self, in_maps):
        return [np.concatenate([np.asarray(m[k]) for m in in_maps], axis=0)
                for k in self.in_names]

    def device_inputs(self, in_maps):
        return [self.jax.device_put(a, self.in_sharding) for a in self.concat_inputs(in_maps)]

    def _zeros(self):
        if self._dev_zeros is None:
            self._dev_zeros = [
                self.jax.device_put(np.zeros((self.n_cores * s[0], *s[1:]), d),
                                    self.in_sharding)
                for s, d in self.zero_shapes]
        return self._dev_zeros

    def execute(self, dev_inputs):
        outs = self.fn(*dev_inputs, *self._zeros())
        return outs

    def run(self, in_maps):
        outs = self.execute(self.device_inputs(in_maps))
        res = []
        for c in range(self.n_cores):
            res.append({name: np.asarray(outs[i]).reshape(self.n_cores, *self.out_avals[i].shape)[c]
                        for i, name in enumerate(self.out_names)})
        return res


_RUNNERS = {}


def _get_runner(T, D, QH, s_wo):
    key = (T, D, QH, s_wo)
    if key not in _RUNNERS:
        _RUNNERS[key] = Runner(_get_nc(T, D, QH, s_wo))
    return _RUNNERS[key]


def kernel(x, r_cos, r_sin, w_norm, wq, wk, wv, wo):
    x = np.asarray(x); r_cos = np.asarray(r_cos); r_sin = np.asarray(r_sin)
    w_norm = np.asarray(w_norm)
    wq = np.asarray(wq); wk = np.asarray(wk); wv = np.asarray(wv); wo = np.asarray(wo)
    T, D = x.shape
    QH = (wq.shape[1] // DH) // NCORES
    runner = _get_runner(T, D, QH, wo_scale(wo))
    in_maps = host_prep(x, r_cos, r_sin, w_norm, wq, wk, wv, wo, T, D, QH)
    return assemble(runner.run(in_maps), T, D)



# revision 51
# speedup vs baseline: 1.2506x; 1.2506x over previous
"""Bass/Trainium2 kernel for LlamaAttention (GQA + RoPE + RMSNorm + causal attention).

Sharding: tensor-parallel over heads across 8 NeuronCores.
  core m: wq columns [m*4h], wk/wv columns [m*1 kv head], wo rows [m*512:...]
  fp8 avn AllGathered across cores; each core computes a wo column slice.

Layout strategy (per core):
  - x shipped pre-transposed/pre-tiled as bf16 -> hT streams as matmul rhs;
    next chunk's window prefetched during attention (ahead of the RS window)
  - q/k produced TRANSPOSED ([dim, token]) straight from weight-stationary
    matmuls; RMSNorm scale s(t) folded into RoPE cos/sin (and into v directly);
    s broadcast across partitions via a K=1 ones-matmul on the PE (no DMA)
  - scoresT[kt,qt] = kT.T @ qT -> exp -> probsT ready as `av` matmul rhs
  - softmax denominators split across engines: even k-blocks summed on the PE
    (PSUM row accumulate), odd blocks on Vector, folded by one f32 matmul;
    reciprocal broadcast via a K=1 ones-matmul on the PE (a DMA bounce here
    would run under the previous chunk's RS and stall the vector queue)
  - wo runs in fp8e4 (e4m3) with DoubleRow perf mode: avn written as
    fp8 (pre-scaled by AVC via the den reciprocal), wo host-quantized with a
    per-tensor scale; dequant is folded into the output residual-add.
    fp8 is confined to wo: quantizing q/k/v pushes rel err past the 2e-2 gate.
  - collective is an AllGather of the fp8 avn (0.25MB/chunk, two head-waves)
    instead of a ReduceScatter of bf16 partials (4MB/chunk); wo is then
    column-sharded per core with full-K f32 PSUM accumulation. The gather
    wait lives on the Pool queue only, and chunk c's wo matmuls are emitted
    after proj(c+1) so their AG dependency has long resolved: no compute
    queue ever head-blocks on collective latency (on the RS design this
    stalled every chunk boundary ~40us).
"""
import sys, math, os

for p in ("/opt/trn_rl_repo", "/root/.axon_site/_ro/trn_rl_repo"):
    if os.path.isdir(p) and p not in sys.path:
        sys.path.insert(0, p)

import numpy as np
import ml_dtypes

import concourse.bass as bass
import concourse.tile as tile
from concourse import bacc, mybir
from concourse.bass_utils import run_bass_kernel_spmd

bf16 = ml_dtypes.bfloat16
F32 = mybir.dt.float32
F32R = mybir.dt.float32r
I32 = mybir.dt.int32
BF16 = mybir.dt.bfloat16
FP8 = mybir.dt.float8e4
fp8np = mybir.dt.np(mybir.dt.float8e4)
Act = mybir.ActivationFunctionType

NCORES = 8
DH = 128          # head dim
EPS = 1e-5
CH = 512          # token chunk
AVC = 32.0        # fp8 scale for normalized attention outputs (absmax ~4.5)


def build(T, D, QH, s_wo):
    """Build the SPMD Bass program. T tokens, D model dim, QH local q heads.
    s_wo: host-side fp8 quant scale of wo (dequant folded into the output stt)."""
    KD = D // 128           # k-chunks over model dim
    NCH = T // CH           # token chunks
    MCH = D // 512          # output-column chunks of wo matmul
    SC = 1.0 / math.sqrt(DH)
    DEQ = 1.0 / (AVC * s_wo)

    nc = bacc.Bacc("TRN2", target_bir_lowering=False, debug=False, num_devices=NCORES)

    # ---- DRAM parameters (per-core shards / replicated) ----
    xarr = nc.dram_tensor("xarr", [128, NCH * KD * 512], BF16, kind="ExternalInput").ap()
    cosP = nc.dram_tensor("cosP", [128, T], F32, kind="ExternalInput").ap()
    sinP = nc.dram_tensor("sinP", [128, T], F32, kind="ExternalInput").ap()
    wqp = nc.dram_tensor("wqp", [128, KD * QH * 128], BF16, kind="ExternalInput").ap()
    wkp = nc.dram_tensor("wkp", [128, KD * 128], BF16, kind="ExternalInput").ap()
    wvp = nc.dram_tensor("wvp", [128, KD * 128], BF16, kind="ExternalInput").ap()
    # wo COLUMN slice for this core: [dh, wave, src_core, h_in_pair, col]
    wop = nc.dram_tensor("wop", [128, 2, NCORES, 2, D // NCORES], FP8,
                         kind="ExternalInput").ap()
    # residual column slice x[:, m*512:(m+1)*512]
    xres = nc.dram_tensor("xres", [T, D // NCORES], F32, kind="ExternalInput").ap()
    tri = nc.dram_tensor("tri", [128, 128], BF16, kind="ExternalInput").ap()
    ident = nc.dram_tensor("ident", [128, 128], BF16, kind="ExternalInput").ap()
    onescol = nc.dram_tensor("onescol", [128, 1], BF16, kind="ExternalInput").ap()
    outp = nc.dram_tensor("out", [T, D // NCORES], F32, kind="ExternalOutput").ap()

    # AllGather-of-avn instead of ReduceScatter-of-partials: ships the fp8
    # avn (0.25MB/chunk) instead of bf16 partials (4MB/chunk); wo becomes
    # column-sharded with full-K f32 PSUM accumulation (no bf16 partial
    # rounding). Head-waves per chunk so the tail only waits for the last
    # head's gather — single-head waves on the last chunk halve the final
    # transfer (the gathered heads land in the two planes of one paired
    # SBUF tile, so the wo DoubleRow pairing is unaffected).
    WAVES = [[(0, 1), (2,), (3,)] for _ in range(NCH)]
    avd = [[nc.dram_tensor(f"avd_{c}_{wi}", [128, len(ws), CH], FP8)
            for wi, ws in enumerate(WAVES[c])] for c in range(NCH)]
    agd = [[nc.dram_tensor(f"agd_{c}_{wi}", [NCORES * 128, len(ws), CH], FP8,
                           addr_space="Shared")
            for wi, ws in enumerate(WAVES[c])] for c in range(NCH)]

    with tile.TileContext(nc) as tc:
        import contextlib
        ctx = contextlib.ExitStack()
        with ctx:
            cpool = ctx.enter_context(tc.tile_pool(name="consts", bufs=1))
            wpool = ctx.enter_context(tc.tile_pool(name="weights", bufs=1))
            xpool = ctx.enter_context(tc.tile_pool(name="xw", bufs=1))
            cs = ctx.enter_context(tc.tile_pool(name="cs", bufs=1))
            work = ctx.enter_context(tc.tile_pool(name="work", bufs=2))
            x2p = ctx.enter_context(tc.tile_pool(name="x2p", bufs=5))
            kvp = ctx.enter_context(tc.tile_pool(name="kvp", bufs=1))
            prp = ctx.enter_context(tc.tile_pool(name="prp", bufs=3))
            evp = ctx.enter_context(tc.tile_pool(name="evp", bufs=3))
            finp = ctx.enter_context(tc.tile_pool(name="finp", bufs=2))
            ps = ctx.enter_context(tc.tile_pool(name="ps", bufs=1, space="PSUM"))

            # ---- constants resident; chunk-0 activations issued BEFORE the
            # 22MB of weights so the PE isn't starved at kernel start ----
            tri_sb = cpool.tile([128, 128], BF16)
            id_sb = cpool.tile([128, 128], BF16)
            ones_sb = cpool.tile([128, 1], BF16)
            # den accumulators use value 1/AVC so the reciprocal comes out as
            # AVC/den, pre-scaling avn into fp8 range at zero extra cost
            onesd_sb = cpool.tile([128, 1], BF16)
            onesd_f32 = cpool.tile([128, 1], F32R)
            onesrow = cpool.tile([1, 128], F32)
            onesrow_r = cpool.tile([1, 128], F32R)
            eps_row = cpool.tile([1, 512], F32)
            zeros_sb = cpool.tile([128, 128], F32)
            ones_dr = cpool.tile([128, 2, 32], FP8)
            nc.vector.memset(ones_dr, 1.0)
            nc.vector.memset(zeros_sb, 0.0)
            nc.vector.memset(onesd_sb, 1.0 / AVC)
            nc.vector.memset(onesrow, 1.0)
            nc.vector.memset(eps_row, EPS)
            # memset can't target f32r; fill via dtype-converting copies
            nc.vector.tensor_copy(onesd_f32, onesd_sb)
            nc.vector.tensor_copy(onesrow_r, onesrow)
            nc.sync.dma_start(out=tri_sb, in_=tri)
            nc.sync.dma_start(out=id_sb, in_=ident)
            nc.sync.dma_start(out=ones_sb, in_=onescol)

            KH = KD // 2  # half window of k-chunks
            # first k-pair split out so the ssq chain starts ~5us earlier
            xw0A = xpool.tile([128, KH * 512], BF16, tag="xwA")
            xw0B = xpool.tile([128, KH * 512], BF16, tag="xwB")
            nc.sync.dma_start(out=xw0A[:, 0:1024], in_=xarr[:, 0:1024])
            nc.sync.dma_start(out=xw0A[:, 1024:KH * 512], in_=xarr[:, 1024:KH * 512])
            nc.sync.dma_start(out=xw0B, in_=xarr[:, KH * 512:KD * 512])
            cos0 = cs.tile([128, 512], F32, tag="cos")
            sin0 = cs.tile([128, 512], F32, tag="sin")
            nc.sync.dma_start(out=cos0, in_=cosP[:, 0:CH])
            nc.sync.dma_start(out=sin0, in_=sinP[:, 0:CH])

            # wq is packed per-head contiguous; load head-by-head (1MB each) so
            # head 0's projection can start ~10us in, before wk/wv/wo arrive
            wq_sb = wpool.tile([128, QH * KD * 128], BF16)
            wk_sb = wpool.tile([128, KD * 128], BF16)
            wv_sb = wpool.tile([128, KD * 128], BF16)
            woc_sb = wpool.tile([128, 2, NCORES, 2, D // NCORES], FP8)
            HW = KD * 128
            for h in range(QH):
                nc.sync.dma_start(out=wq_sb[:, h * HW:(h + 1) * HW],
                                  in_=wqp[:, h * HW:(h + 1) * HW])
            nc.sync.dma_start(out=wk_sb, in_=wkp)
            nc.sync.dma_start(out=wv_sb, in_=wvp)
            nc.sync.dma_start(out=woc_sb, in_=wop)

            # residual column-slice tiles: loaded per chunk (consumed by
            # wo_block(c) during chunk c+1), rotating 2 buffers per tb slot
            xr_t = {}

            def load_xr(cc):
                for tb in range(4):
                    i = cc * 4 + tb
                    xr_sb = finp.tile([128, D // NCORES], F32, tag=f"xr{tb}",
                                      bufs=2)
                    nc.sync.dma_start(out=xr_sb, in_=xres[i * 128:(i + 1) * 128, :])
                    xr_t[(cc, tb)] = xr_sb

            # persistent k/v for the whole sequence
            kT_sb = kvp.tile([128, T], BF16)   # [dim, token], roped+normed
            v_sb = kvp.tile([128, T], BF16)    # v natural: block j at cols j*128

            def prefetch_x(c):
                """Issue chunk c's x-window/cos/sin DMAs. Called one chunk
                ahead (right after proj(c-1)) so the 4.2MB window is resident
                before the previous chunk's ReduceScatter contends for DMA."""
                csl = bass.ds(c * CH, CH)
                xwA = xpool.tile([128, KH * 512], BF16, tag="xwA")
                xwB = xpool.tile([128, KH * 512], BF16, tag="xwB")
                nc.sync.dma_start(out=xwA, in_=xarr[:, (c * KD) * 512:(c * KD + KH) * 512])
                nc.sync.dma_start(out=xwB, in_=xarr[:, (c * KD + KH) * 512:(c * KD + KD) * 512])
                cos_sl = cs.tile([128, 512], F32, tag="cos")
                sin_sl = cs.tile([128, 512], F32, tag="sin")
                nc.sync.dma_start(out=cos_sl, in_=cosP[:, csl])
                nc.sync.dma_start(out=sin_sl, in_=sinP[:, csl])
                return xwA, xwB, cos_sl, sin_sl

            def norm_rest(c, tiles):
                """rmsnorm scale + rope tables for chunk c (at chunk start)."""
                xwA, xwB, cos_sl, sin_sl = tiles

                def xw(k):
                    buf = xwA if k < KH else xwB
                    kk = k % KH
                    return buf[:, kk * 512:(kk + 1) * 512]

                # x^2 in fp8e4 so the ssq reduction runs DoubleRow (0.5 cyc/row
                # instead of 1): x^2 <= ~30 fits e4m3 easily, and the per-term
                # ~4% quantization noise averages to <0.1% over the 4096-sum.
                # Engines still alternate (vector even plane, scalar odd).
                ssq_ps = ps.tile([32, 512], F32, tag="row", bufs=1)
                for i in range(KD // 2):
                    x2 = x2p.tile([128, 2, 512], FP8, tag="x2")
                    nc.vector.tensor_mul(x2[:, 0, :], xw(2 * i), xw(2 * i))
                    if c == 0 and i == 0:
                        # keep Scalar's one-time ACT_TABLE load off the
                        # kernel-start critical path
                        nc.vector.tensor_mul(x2[:, 1, :], xw(1), xw(1))
                    else:
                        nc.scalar.activation(x2[:, 1, :], xw(2 * i + 1), Act.Square)
                    nc.tensor.matmul(ssq_ps, ones_dr, x2,
                                     start=(i == 0), stop=(i == KD // 2 - 1),
                                     perf_mode=mybir.MatmulPerfMode.DoubleRow)
                # rsqrt entirely on DVE (magic-constant seed + 2 Newton steps).
                # Keeps Sqrt off the Scalar engine: sqrt lives in a different
                # activation-table set than exp/square/copy, so using it forced
                # two ~1.3us ACT_TABLE_LOADs per chunk right in the exp path.
                # The seed constant math runs in fp32 VALUE space (DVE add/mult
                # fp-cast int operands anyway); 2^-24 rounding on the int is
                # far below the seed's own 3.4% error.
                z_sb = work.tile([1, 512], F32, tag="sq", bufs=1)
                nc.vector.scalar_tensor_tensor(
                    out=z_sb[0:1, :], in0=ssq_ps[0:1, :], scalar=1.0 / D,
                    in1=eps_row[0:1, :], op0=mybir.AluOpType.mult,
                    op1=mybir.AluOpType.add)
                # s_sb is float32r so its producer "rounds to FP32r" (BIR
                # verifier requirement for f32r matmul operands)
                s_sb = work.tile([1, 512], F32R, tag="s", bufs=1)
                u_sb = work.tile([1, 512], F32, tag="u", bufs=1)
                w_sb = work.tile([1, 512], I32, tag="wi", bufs=1)
                # u = float(bits(z)); u = C - 0.5*u (seed bits as float value);
                # w = int(u); s = bitcast(w). s_sb only ever sees f32r-typed
                # writes (BIR verifier: f32r matmul operands need f32r writers)
                nc.vector.tensor_copy(u_sb[0:1, :], z_sb[0:1, :].bitcast(I32))
                nc.vector.tensor_scalar(
                    out=u_sb[0:1, :], in0=u_sb[0:1, :], scalar1=-0.5,
                    scalar2=float(0x5F3759DF), op0=mybir.AluOpType.mult,
                    op1=mybir.AluOpType.add)
                nc.vector.tensor_copy(w_sb[0:1, :], u_sb[0:1, :])
                nc.vector.tensor_copy(s_sb[0:1, :], w_sb[0:1, :].bitcast(F32))
                for _ in range(2):
                    nc.vector.tensor_mul(u_sb[0:1, :], s_sb[0:1, :], s_sb[0:1, :])
                    nc.vector.tensor_mul(u_sb[0:1, :], u_sb[0:1, :], z_sb[0:1, :])
                    nc.vector.tensor_scalar(
                        out=u_sb[0:1, :], in0=u_sb[0:1, :], scalar1=-0.5,
                        scalar2=1.5, op0=mybir.AluOpType.mult,
                        op1=mybir.AluOpType.add)
                    nc.vector.tensor_mul(s_sb[0:1, :], s_sb[0:1, :], u_sb[0:1, :])
                # broadcast partition 0 -> 128 ON THE PE (K=1 ones-matmul into
                # PSUM): no DMA involved, so the chunk-start rope chain never
                # races a ReduceScatter for DMA bandwidth. f32r: 1 cyc/row on
                # the PE instead of fp32's 4.
                s_ps = ps.tile([128, 512], F32, tag="sbc", bufs=1)
                nc.tensor.matmul(s_ps, onesrow_r[0:1, :], s_sb[0:1, :],
                                 start=True, stop=True)
                s_bc = work.tile([128, 512], F32, tag="s_bc")
                nc.vector.tensor_copy(s_bc, s_ps)

                cosn = cs.tile([128, 512], BF16, tag="cosn")
                sinn = cs.tile([128, 512], BF16, tag="sinn")
                nc.vector.tensor_mul(cosn, cos_sl, s_ps)
                nc.vector.tensor_mul(sinn, sin_sl, s_ps)
                return xw, s_bc, cosn, sinn

            avf_t = {}

            def wo_block(cc, interleave=False):
                """Column-sharded wo for chunk cc from the gathered fp8 avn:
                16 DoubleRow matmuls per 128-token block (full K=4096 in f32
                PSUM), then one fused dequant+residual stt and the store.
                interleave=True (last chunk): run every block's wave-0 half
                first so the PE chews through it while AG(wave 1) is still in
                flight — needs 4 concurrent PSUM groups (3 acc + 1 scores)."""
                def mm(wo_ps, w, j, tb, start, stop):
                    nc.tensor.matmul(
                        wo_ps,
                        avf_t[(w, j)][0][:, :, tb * 128:(tb + 1) * 128],
                        woc_sb[:, w, j, :, :],
                        start=start, stop=stop,
                        perf_mode=mybir.MatmulPerfMode.DoubleRow)

                def evac(wo_ps, tb):
                    o_sb = evp.tile([128, 512], F32, tag="osb")
                    i = cc * 4 + tb
                    nc.vector.scalar_tensor_tensor(
                        out=o_sb, in0=wo_ps, scalar=DEQ, in1=xr_t[(cc, tb)],
                        op0=mybir.AluOpType.mult, op1=mybir.AluOpType.add)
                    nc.gpsimd.dma_start(out=outp[i * 128:(i + 1) * 128, :],
                                        in_=o_sb)

                if not interleave:
                    for tb in range(4):
                        wo_ps = ps.tile([128, 512], F32, tag="acc", bufs=3)
                        for n in range(2 * NCORES):
                            mm(wo_ps, n // NCORES, n % NCORES, tb,
                               n == 0, n == 2 * NCORES - 1)
                        evac(wo_ps, tb)
                else:
                    psums = []
                    for tb in range(4):
                        wo_ps = ps.tile([128, 512], F32,
                                        tag="acc" if tb < 3 else "scores",
                                        bufs=3 if tb < 3 else 2)
                        psums.append(wo_ps)
                        for j in range(NCORES):
                            mm(wo_ps, 0, j, tb, j == 0, False)
                    for tb in range(4):
                        for j in range(NCORES):
                            mm(psums[tb], 1, j, tb, False, j == NCORES - 1)
                        evac(psums[tb], tb)

            nxt_x = (xw0A, xw0B, cos0, sin0)

            for c in range(NCH):
                csl = bass.ds(c * CH, CH)
                xw, s_bc, cosn, sinn = norm_rest(c, nxt_x)

                # ---- projections (weight-stationary, transposed outputs) ----
                def rope_evac(pp, dest):
                    # dest = pp * cosn + swap64(pp) * sinn   (sign baked into sinn)
                    t1 = work.tile([128, 512], F32, tag="t1")
                    t2 = work.tile([128, 512], F32, tag="t2")
                    nc.vector.tensor_mul(t1, pp, cosn)
                    nc.vector.tensor_mul(t2[0:64, :], pp[64:128, :], sinn[0:64, :])
                    nc.vector.tensor_mul(t2[64:128, :], pp[0:64, :], sinn[64:128, :])
                    nc.vector.tensor_add(dest, t1, t2)

                qT_all = work.tile([128, QH * 512], BF16, tag="qT")
                for h in range(QH):
                    pp = ps.tile([128, 512], F32, tag="acc", bufs=3)
                    for k in range(KD):
                        nc.tensor.matmul(pp, wq_sb[:, (h * KD + k) * 128:(h * KD + k + 1) * 128],
                                         xw(k), start=(k == 0), stop=(k == KD - 1))
                    rope_evac(pp, qT_all[:, h * 512:(h + 1) * 512])
                pp = ps.tile([128, 512], F32, tag="acc", bufs=3)
                for k in range(KD):
                    nc.tensor.matmul(pp, wk_sb[:, k * 128:(k + 1) * 128], xw(k),
                                     start=(k == 0), stop=(k == KD - 1))
                rope_evac(pp, kT_sb[:, csl])
                pp = ps.tile([128, 512], F32, tag="acc", bufs=3)
                for k in range(KD):
                    nc.tensor.matmul(pp, wv_sb[:, k * 128:(k + 1) * 128], xw(k),
                                     start=(k == 0), stop=(k == KD - 1))
                vtmp = work.tile([128, 512], BF16, tag="vtmp")
                nc.vector.tensor_mul(vtmp, pp, s_bc)
                for tb in range(4):
                    trp = ps.tile([128, 128], BF16, tag="tr", bufs=1)
                    nc.tensor.transpose(trp, vtmp[:, tb * 128:(tb + 1) * 128], id_sb)
                    nc.vector.tensor_copy(v_sb[:, (4 * c + tb) * 128:(4 * c + tb + 1) * 128], trp)

                # prefetch next chunk's x window now (its WAR on proj(c) clears
                # exactly as proj drains; arrives during attention, before the
                # AG window could starve it)
                if c + 1 < NCH:
                    nxt_x = prefetch_x(c + 1)
                load_xr(c)

                # ---- wo for the PREVIOUS chunk: emitted here (after proj(c),
                # before attention(c)) so its PE-queue wait on AG(c-1) has long
                # resolved by the time the PE reaches it ----
                if c > 0:
                    wo_block(c - 1)

                # ---- attention for this chunk's 512 query tokens ----
                NJ = 4 * (c + 1)
                avn_all = work.tile([128, QH, 512], FP8, tag="avn")
                for h in range(QH):
                    av_ps = ps.tile([128, 512], F32, tag="acc", bufs=3)
                    # den split across engines to balance the attention inner
                    # loop: even j summed on PE (PSUM accumulate), odd j on
                    # Vector (SBUF f32 accumulate), folded together at the end.
                    den_ps = ps.tile([1, 512], F32, tag="row", bufs=1)
                    # f32r so the fold matmul below gets the fast PE path
                    den_acc = work.tile([128, 512], F32R, tag="den_acc")
                    if c == 0:
                        # j=1 (first vector-side block) is diagonal here; its
                        # [0:128] region is never written, zero it for the fold
                        # (copy from a zero tile: memset can't target f32r)
                        nc.vector.tensor_copy(den_acc[:, 0:128], zeros_sb)
                    qTh = qT_all[:, h * 512:(h + 1) * 512]
                    for j in range(NJ):
                        sc_ps = ps.tile([128, 512], F32, tag="scores", bufs=2)
                        nc.tensor.matmul(sc_ps, kT_sb[:, j * 128:(j + 1) * 128], qTh,
                                         start=True, stop=True)
                        pr = prp.tile([128, 512], BF16, tag="pr")
                        off = max(0, 128 * (j - 4 * c))
                        nc.scalar.activation(pr[:, off:512], sc_ps[:, off:512],
                                             Act.Exp, scale=SC)
                        if j >= 4 * c:
                            nc.vector.tensor_mul(pr[:, off:off + 128], pr[:, off:off + 128], tri_sb)
                        if j % 2 == 0:
                            nc.tensor.matmul(den_ps[0:1, off:512], onesd_sb, pr[:, off:512],
                                             start=(j == 0), stop=False)
                        elif j == 1:
                            nc.vector.tensor_copy(den_acc[:, off:512], pr[:, off:512])
                        else:
                            nc.vector.tensor_add(den_acc[:, off:512], den_acc[:, off:512],
                                                 pr[:, off:512])
                        nc.tensor.matmul(av_ps[:, off:512], v_sb[:, j * 128:(j + 1) * 128],
                                         pr[:, off:512], start=(j == 0), stop=(j == NJ - 1))
                    # fold the vector-side partial into the PSUM row and finish
                    # (f32r moving operand: 1 cyc/row instead of fp32's 4)
                    nc.tensor.matmul(den_ps, onesd_f32, den_acc, start=False, stop=True)
                    den_r = work.tile([1, 512], F32, tag="den_r")
                    nc.vector.reciprocal_approx_fast(out=den_r[0:1, :], in_=den_ps[0:1, :])
                    # broadcast the reciprocal on the PE (no DMA bounce). The
                    # f32r round-trip (reciprocal must write f32) costs one
                    # small DVE copy but drops the matmul from 4 cyc/row to 1.
                    den_rr = work.tile([1, 512], F32R, tag="den_rr", bufs=1)
                    nc.vector.tensor_copy(den_rr[0:1, :], den_r[0:1, :])
                    den_bc = ps.tile([128, 512], F32, tag="sbc", bufs=1)
                    nc.tensor.matmul(den_bc, onesrow_r[0:1, :], den_rr[0:1, :],
                                     start=True, stop=True)
                    avs = work.tile([128, 512], BF16, tag="avs")
                    nc.vector.tensor_copy(avs, av_ps)
                    nc.vector.tensor_mul(avn_all[:, h, :], avs, den_bc)

                    # ship completed waves: write fp8 avn to DRAM, AllGather,
                    # stage the 8 cores' slices back into paired SBUF tiles.
                    # Pool-queue ops (the gather wait never blocks compute
                    # queues); the final wave's loads split across Pool+Sync
                    # so their serial issue doesn't pace the tail.
                    for wi, ws in enumerate(WAVES[c]):
                        if h != ws[-1]:
                            continue
                        nc.sync.dma_start(
                            out=avd[c][wi].ap(),
                            in_=avn_all[:, ws[0]:ws[-1] + 1, :])
                        nc.gpsimd.collective_compute(
                            "AllGather", mybir.AluOpType.bypass,
                            replica_groups=[list(range(NCORES))],
                            ins=[avd[c][wi].ap()], outs=[agd[c][wi].ap()])
                        last_wave = wi == len(WAVES[c]) - 1
                        for j in range(NCORES):
                            for e, hh in enumerate(ws):
                                p = hh // 2
                                if (p, j) not in avf_t or avf_t[(p, j)][1] != c:
                                    avf_t[(p, j)] = (finp.tile(
                                        [128, 2, CH], FP8, tag=f"avf{p}_{j}",
                                        bufs=1, name=f"avf{p}_{j}"), c)
                                t = avf_t[(p, j)][0]
                                eng = nc.sync if (last_wave and c == NCH - 1
                                                  and j >= 4) else nc.gpsimd
                                eng.dma_start(
                                    out=t[:, hh % 2:hh % 2 + 1, :],
                                    in_=agd[c][wi].ap()[j * 128:(j + 1) * 128,
                                                        e:e + 1, :])

                if c == NCH - 1:
                    wo_block(c, interleave=True)

    nc.compile()
    return nc


# host-side permutation: de-interleave rope pairs (2i, 2i+1) -> (i, 64+i)
_PERM = np.concatenate([np.arange(0, DH, 2), np.arange(1, DH, 2)])


def wo_scale(wo):
    return 240.0 / max(float(np.abs(wo).max()), 1e-30)


def host_prep(x, r_cos, r_sin, w_norm, wq, wk, wv, wo, T, D, QH):
    """Build per-core input maps."""
    KD = D // 128
    NCH = T // CH
    NH = wq.shape[1] // DH
    NKV = wk.shape[1] // DH
    s_wo = wo_scale(wo)

    xT = np.ascontiguousarray(x.T)  # [D, T]
    xarr = np.ascontiguousarray(
        xT.reshape(KD, 128, NCH, 512).transpose(1, 2, 0, 3)).reshape(128, NCH * KD * 512)
    xarr = xarr.astype(bf16)

    cosP = np.ascontiguousarray(r_cos.T[_PERM, :]).astype(np.float32)
    sinP = np.ascontiguousarray(r_sin.T[_PERM, :]).astype(np.float32)
    sinP[:64, :] *= -1.0

    wn = w_norm[:, None].astype(np.float32)
    wq_p = (wq * wn).reshape(D, NH, DH)[:, :, _PERM].reshape(D, NH * DH)
    wk_p = (wk * wn).reshape(D, NKV, DH)[:, :, _PERM].reshape(D, NKV * DH)
    wv_p = wv * wn

    tri_m = (np.arange(128)[:, None] <= np.arange(128)[None, :]).astype(bf16)  # kt <= qt
    ident = np.eye(128, dtype=bf16)
    onescol = np.ones((128, 1), dtype=bf16)

    S = CH // NCORES  # 64 rows per core per chunk

    in_maps = []
    for m in range(NCORES):
        wq_m = wq_p[:, m * QH * 128:(m + 1) * QH * 128]
        # head-major packing: [128][QH][KD][128] so each head's weights are a
        # contiguous 1MB DMA
        wq_m = np.ascontiguousarray(
            wq_m.reshape(KD, 128, QH, 128).transpose(1, 2, 0, 3)).reshape(128, QH * KD * 128)
        wk_m = wk_p[:, m * 128:(m + 1) * 128]
        wk_m = np.ascontiguousarray(
            wk_m.reshape(KD, 128, 128).transpose(1, 0, 2)).reshape(128, KD * 128)
        wv_m = wv_p[:, m * 128:(m + 1) * 128]
        wv_m = np.ascontiguousarray(
            wv_m.reshape(KD, 128, 128).transpose(1, 0, 2)).reshape(128, KD * 128)
        # wo COLUMN slice for this core, rows regrouped as
        # [dh, wave, src_core, h_in_pair, col] to match the DR pairing of the
        # gathered avn (src core j's heads 2w/2w+1 are dim rows (j*4+2w+e)*128+dh)
        wo_m = wo[:, m * (D // NCORES):(m + 1) * (D // NCORES)]
        wo_m = np.ascontiguousarray(
            wo_m.reshape(NCORES, 2, 2, 128, D // NCORES).transpose(3, 1, 0, 2, 4))
        wo_m = np.clip(wo_m * s_wo, -240.0, 240.0).astype(fp8np)

        # residual column slice, rows already grouped per (chunk, token-block)
        xres_m = np.ascontiguousarray(
            x[:, m * (D // NCORES):(m + 1) * (D // NCORES)]).astype(np.float32)

        in_maps.append({
            "xarr": xarr, "cosP": cosP, "sinP": sinP,
            "wqp": wq_m.astype(bf16), "wkp": wk_m.astype(bf16),
            "wvp": wv_m.astype(bf16), "wop": wo_m,
            "xres": np.ascontiguousarray(xres_m),
            "tri": tri_m, "ident": ident, "onescol": onescol,
        })
    return in_maps


def assemble(results, T, D):
    # each core produced its full-height output column slice
    out = np.empty((T, D), np.float32)
    for m in range(NCORES):
        out[:, m * (D // NCORES):(m + 1) * (D // NCORES)] = results[m]["out"]
    return out


_CACHE = {}


def _get_nc(T, D, QH, s_wo):
    key = (T, D, QH, s_wo)
    if key not in _CACHE:
        _CACHE[key] = build(T, D, QH, s_wo)
    return _CACHE[key]


class Runner:
    """Cached-jit SPMD runner (replicates bass2jax.run_bass_via_pjrt but reuses the
    jitted callable across calls and supports device-resident inputs for timing)."""

    def __init__(self, nc, n_cores=NCORES):
        import jax
        from jax.experimental.shard_map import shard_map
        from jax.sharding import Mesh, PartitionSpec, NamedSharding
        from concourse import bass2jax
        bass2jax.install_neuronx_cc_hook()
        self.jax = jax
        self.nc = nc
        self.n_cores = n_cores
        partition_name = nc.partition_id_tensor.name if nc.partition_id_tensor else None
        in_names, out_names, out_avals, zero_shapes = [], [], [], []
        for alloc in nc.m.functions[0].allocations:
            if not isinstance(alloc, mybir.MemoryLocationSet):
                continue
            name = alloc.memorylocations[0].name
            if alloc.kind == "ExternalInput":
                if name != partition_name:
                    in_names.append(name)
            elif alloc.kind == "ExternalOutput":
                out_names.append(name)
                shape = tuple(alloc.tensor_shape)
                dtype = mybir.dt.np(alloc.dtype)
                out_avals.append(jax.core.ShapedArray(shape, dtype))
                zero_shapes.append((shape, dtype))
        self.in_names, self.out_names = in_names, out_names
        self.out_avals, self.zero_shapes = out_avals, zero_shapes
        n_params, n_outs = len(in_names), len(out_names)
        all_names = in_names + out_names
        if partition_name is not None:
            all_names = all_names + [partition_name]

        def _body(*args):
            operands = list(args)
            if partition_name is not None:
                operands.append(bass2jax.partition_id_tensor())
            outs = bass2jax._bass_exec_p.bind(
                *operands,
                out_avals=tuple(out_avals),
                in_names=tuple(all_names),
                out_names=tuple(out_names),
                lowering_input_output_aliases=(),
                sim_require_finite=True,
                sim_require_nnan=True,
                nc=nc,
            )
            return tuple(outs)

        devices = jax.devices()[:n_cores]
        self.mesh = Mesh(np.asarray(devices), ("core",))
        self.in_sharding = NamedSharding(self.mesh, PartitionSpec("core"))
        in_specs = (PartitionSpec("core"),) * (n_params + n_outs)
        out_specs = (PartitionSpec("core"),) * n_outs
        # No donation: the kernel writes every ExternalOutput element, so the
        # "zero" output operands can stay device-resident and be reused across
        # calls instead of being re-uploaded (32 MB/call through the tunnel).
        self.fn = jax.jit(
            shard_map(_body, mesh=self.mesh, in_specs=in_specs,
                      out_specs=out_specs, check_rep=False),
            keep_unused=True)
        self._dev_zeros = None

    def concat_inputs(self, in_maps):
        return [np.concatenate([np.asarray(m[k]) for m in in_maps], axis=0)
                for k in self.in_names]

    def device_inputs(self, in_maps):
        return [self.jax.device_put(a, self.in_sharding) for a in self.concat_inputs(in_maps)]

    def _zeros(self):
        if self._dev_zeros is None:
            self._dev_zeros = [
                self.jax.device_put(np.zeros((self.n_cores * s[0], *s[1:]), d),
                                    self.in_sharding)
                for s, d in self.zero_shapes]
        return self._dev_zeros

    def execute(self, dev_inputs):
        outs = self.fn(*dev_inputs, *self._zeros())
        return outs

    def run(self, in_maps):
        outs = self.execute(self.device_inputs(in_maps))
        res = []
        for c in range(self.n_cores):
            res.append({name: np.asarray(outs[i]).reshape(self.n_cores, *self.out_avals[i].shape)[c]
                        for i, name in enumerate(self.out_names)})
        return res


_RUNNERS = {}


def _get_runner(T, D, QH, s_wo):
    key = (T, D, QH, s_wo)
    if key not in _RUNNERS:
        _RUNNERS[key] = Runner(_get_nc(T, D, QH, s_wo))
    return _RUNNERS[key]


def kernel(x, r_cos, r_sin, w_norm, wq, wk, wv, wo):
    x = np.asarray(x); r_cos = np.asarray(r_cos); r_sin = np.asarray(r_sin)
    w_norm = np.asarray(w_norm)
    wq = np.asarray(wq); wk = np.asarray(wk); wv = np.asarray(wv); wo = np.asarray(wo)
    T, D = x.shape
    QH = (wq.shape[1] // DH) // NCORES
    runner = _get_runner(T, D, QH, wo_scale(wo))
    in_maps = host_prep(x, r_cos, r_sin, w_norm, wq, wk, wv, wo, T, D, QH)
    return assemble(runner.run(in_maps), T, D)

